# revision 14
# baseline (speedup 1.0000x reference)
"""Trainium2 Bass kernel: polar/cartesian ConvNext feature mix + 25-head scan.

Full (unsharded) inputs in, full output out. Pure data-parallel over batch
(32 -> 4 per core x 8 cores).

v2 design (validated vs the jax reference at ~1e-2 rel in numpy sim):
  * polar stream: int8 with per-core per-(channel,ring) scales in HBM;
    the width-256 mean starts as a fold-2 SWDGE cast+accum DMA (int8
    pairs sum exactly in f16), then a flat in-place f16 halving tree on
    the DVE (w-major host layout keeps every level a packed 2-d
    TensorTensor eligible for the 2x mode). The dequant scales and the
    /256 fold into a per-core bf16 W1 (bf16 dodges f16 subnormals), so
    fe holds raw integer sums.
  * cart stream: fp8-e3m4 in HBM, consumed directly by the PE as the
    moving operand against a bf16 smat (grid-sample weight matrix, /256
    folded in) built host-side from `grid`.
  * head linear: per-ring [128,4]x[128,40] matmuls into [4,40] PSUM
    groups (engine APs require 32-aligned partition bases, so batched
    100-partition layouts are not expressible); b1_eff enters each group
    as a K=1 ones-row matmul; the cart half runs during the polar stream
    with scalar-engine PSUM evacuation, the polar half merges at the end.
  * the sequential 25-step head recurrence is replaced by Jacobi
    iteration on [4, 25, 40] tiles: the recurrent term's gain is ~1.3e-3
    per step, so pass-0 + one correction pass is numerically exact; the
    ring-shift of the carried scalar is a free-dim shifted copy.
    gelu(exact) ~= 0.5*x*(1+tanh(c*x)).
  * engine assignment: cart/const DMAs on sync (HWDGE), polar DMAs on
    gpsimd (SWDGE), PSUM evacuation copies on the scalar engine (its
    in-order stream absorbs the PE-wait stalls), all tree/scale/Jacobi
    math on the DVE, which must stay free of PE-dependent stalls.
"""
import numpy as np
import ml_dtypes

import concourse.bacc as bacc
import concourse.mybir as mybir
import concourse.tile as tile
from concourse import bass_utils
from concourse.masks import make_identity

F32 = mybir.dt.float32
F16 = mybir.dt.float16
BF16 = mybir.dt.bfloat16
F8E3 = mybir.dt.float8e3
I8 = mybir.dt.int8
AF = mybir.ActivationFunctionType
ALU = mybir.AluOpType
AX = mybir.AxisListType

# Problem shapes (fixed by the spec)
B, C, RHO, WP = 32, 384, 25, 256
HC = WC = 64
NPIX = HC * WC            # 4096
D = 2 * C                 # 768
NH = 40
NCORES = 8
BPC = B // NCORES         # 4
CCH = C // 128            # 3 channel chunks
KCH = NPIX // 128         # 32 pixel chunks
DCH = D // 128            # 6 feature chunks
KHALF = KCH // 2
NJAC = 1                  # jacobi correction passes

GC = 0.7978845608028654   # sqrt(2/pi)

TRACE = False             # test harness may flip this for profiling
TRACE_KW: dict = {}
LAST_RESULTS = None


def _build_smat(grid):
    """[B, 4096, 25] f32: summed bilinear weights per (pixel, ring).

    Index math replicates the reference exactly (f32 floor/clip)."""
    gx = grid[..., 0].astype(np.float32)
    gy = grid[..., 1].astype(np.float32)
    ix = (gx + np.float32(1.0)) * np.float32(WC * 0.5) - np.float32(0.5)
    iy = (gy + np.float32(1.0)) * np.float32(HC * 0.5) - np.float32(0.5)
    ix0 = np.floor(ix)
    iy0 = np.floor(iy)
    tx = ix - ix0
    ty = iy - iy0
    corners = (
        (ix0, iy0, (1 - tx) * (1 - ty)),
        (ix0 + 1, iy0, tx * (1 - ty)),
        (ix0, iy0 + 1, (1 - tx) * ty),
        (ix0 + 1, iy0 + 1, tx * ty),
    )
    boff = np.arange(B, dtype=np.int64)[:, None, None] * (NPIX * RHO)
    roff = np.arange(RHO, dtype=np.int64)[None, :, None]
    keys = []
    vals = []
    for xi, yi, w in corners:
        valid = (xi >= 0) & (xi < WC) & (yi >= 0) & (yi < HC)
        xc = np.clip(xi, 0, WC - 1).astype(np.int64)
        yc = np.clip(yi, 0, HC - 1).astype(np.int64)
        keys.append((boff + (yc * WC + xc) * RHO + roff).ravel())
        vals.append((w * valid).astype(np.float64).ravel())
    s = np.bincount(np.concatenate(keys), weights=np.concatenate(vals),
                    minlength=B * NPIX * RHO)
    return s.reshape(B, NPIX, RHO).astype(np.float32)


def _build_program():
    nc = bacc.Bacc("TRN2", target_bir_lowering=False, debug=False,
                   enable_asserts=False, num_devices=NCORES)
    polar = nc.dram_tensor("polar", [BPC, 128, CCH, WP * RHO], I8,
                           kind="ExternalInput")
    cart = nc.dram_tensor("cart", [BPC, 128, KCH, C], F8E3,
                          kind="ExternalInput")
    smat = nc.dram_tensor("smat", [128, BPC, KCH, RHO], BF16,
                          kind="ExternalInput")
    w1 = nc.dram_tensor("w1", [128, DCH, RHO, NH], BF16, kind="ExternalInput")
    b1b = nc.dram_tensor("b1b", [1, RHO, NH], F32, kind="ExternalInput")
    wrecb = nc.dram_tensor("wrecb", [BPC, RHO, NH], F32, kind="ExternalInput")
    w2hb = nc.dram_tensor("w2hb", [BPC, RHO, NH], F32, kind="ExternalInput")
    b2b = nc.dram_tensor("b2b", [BPC, RHO], F32, kind="ExternalInput")
    out = nc.dram_tensor("out", [BPC, RHO], F32, kind="ExternalOutput")

    with tile.TileContext(nc) as tc:
        with (
            tc.tile_pool(name="sing", bufs=1) as sing,
            tc.tile_pool(name="cpool", bufs=3) as cpool,
            tc.tile_pool(name="ppool", bufs=3) as ppool,
            tc.tile_pool(name="fpool", bufs=4) as fpool,
            tc.tile_pool(name="fcpool", bufs=2) as fcpool,
            tc.tile_pool(name="scanw", bufs=1) as scanw,
            tc.tile_pool(name="cps", bufs=2, space="PSUM") as cps,
            tc.tile_pool(name="tps", bufs=2, space="PSUM") as tps,
            tc.tile_pool(name="hps", bufs=3, space="PSUM") as hps,
        ):
            stile = sing.tile([128, BPC, KCH, RHO], BF16)
            w1_sb = sing.tile([128, DCH, RHO, NH], BF16)
            fe_sb = sing.tile([128, DCH, RHO, BPC], BF16)
            ident = sing.tile([RHO, RHO], F32)
            b1b_sb = sing.tile([1, RHO, NH], F32)
            wrecb_sb = sing.tile([BPC, RHO, NH], F32)
            w2hb_sb = sing.tile([BPC, RHO, NH], F32)
            b2b_sb = sing.tile([BPC, RHO], F32)
            onesc = sing.tile([1, BPC], F32)
            hpre_sb = sing.tile([BPC, RHO, NH], F32)
            pre_sb = sing.tile([BPC, RHO, NH], F32)
            accsh = sing.tile([BPC, RHO, 1], F32)
            warm = sing.tile([BPC, RHO], F32)

            def load_consts():
                make_identity(nc, ident)
                nc.sync.dma_start(out=stile, in_=smat.ap())
                nc.sync.dma_start(out=w1_sb, in_=w1.ap())
                nc.sync.dma_start(out=b1b_sb, in_=b1b.ap())
                nc.sync.dma_start(out=wrecb_sb, in_=wrecb.ap())
                nc.sync.dma_start(out=w2hb_sb, in_=w2hb.ap())
                nc.sync.dma_start(out=b2b_sb, in_=b2b.ap())
                nc.vector.memset(onesc, 1.0)
                # warm the tanh LUT while streaming
                nc.scalar.activation(out=warm, in_=b2b_sb, func=AF.Tanh,
                                     scale=1.0)

            # polar lands via fold-2 SWDGE cast+accum DMAs: the second
            # half-width accumulates onto the first in f16 (sums <=254 are
            # exact), halving the on-chip tree work. batch 0 per-chunk so
            # its trees start early
            HW = WP * RHO // 2
            pts = []
            for b in range(BPC):
                pt = ppool.tile([128, CCH, WP * RHO], I8, tag="p",
                                name=f"p{b}")
                if b == 0:
                    for cc in range(CCH):
                        nc.gpsimd.dma_start(out=pt[:, cc, :],
                                            in_=polar.ap()[b][:, cc])
                else:
                    nc.gpsimd.dma_start(out=pt, in_=polar.ap()[b])
                pts.append(pt)

            # cart stream + matmuls + transposes (PE) / evacuations (ACT)
            for b in range(BPC):
                ctl = cpool.tile([128, KCH, C], F8E3, tag="c")
                nc.sync.dma_start(out=ctl[:, 0:KHALF, :],
                                  in_=cart.ap()[b][:, 0:KHALF, :])
                nc.sync.dma_start(out=ctl[:, KHALF:KCH, :],
                                  in_=cart.ap()[b][:, KHALF:KCH, :])
                if b == 0:
                    load_consts()
                cpsum = cps.tile([RHO, C], F32, tag="cp", name=f"cp{b}")
                for k in range(KCH):
                    nc.tensor.matmul(cpsum, stile[:, b, k, :], ctl[:, k, :],
                                     start=(k == 0), stop=(k == KCH - 1))
                fecart = fcpool.tile([RHO, C], F32, tag="fc", name=f"fc{b}")
                nc.scalar.copy(out=fecart, in_=cpsum)
                for cc in range(CCH):
                    tp = tps.tile([128, RHO], F32, tag="tp",
                                  name=f"tp{b}_{cc}")
                    nc.tensor.transpose(
                        tp, fecart[:, cc * 128:(cc + 1) * 128], ident)
                    nc.scalar.copy(out=fe_sb[:, CCH + cc, :, b], in_=tp)

            # cart half of the head linear: per-ring [4,40] groups with
            # the b1_eff bias entering as a K=1 ones-row matmul; evacuated
            # by the scalar engine so the DVE stream stays stall-free
            for r in range(RHO):
                hp = hps.tile([BPC, NH], F32, tag="hp", name=f"hpC{r}")
                nc.tensor.matmul(hp, onesc, b1b_sb[:, r, :],
                                 start=True, stop=False)
                for kk in range(CCH, DCH):
                    nc.tensor.matmul(hp, fe_sb[:, kk, r, :],
                                     w1_sb[:, kk, r, :],
                                     start=False, stop=(kk == DCH - 1))
                nc.scalar.copy(out=hpre_sb[:, r, :], in_=hp)

            # polar reduction trees: flat contiguous in-place halving in
            # f16 (2-d packed TensorTensor hits the DVE 2x mode); the
            # final level writes the raw ring sums into fe_sb (the int8
            # dequant scales are folded into the bf16 W1 host-side)
            # level 1 (int8, always 1x) is the expensive level: offload a
            # few tiles' L1 to the otherwise-idle gpsimd engine; the f16
            # levels (2x-eligible) stay on the DVE
            GP_L1 = {(0, 1), (1, 0), (2, 0), (3, 0)}

            def tree(l1_eng, pt, cc, b):
                ft = fpool.tile([128, HW], F16, tag="f")
                l1_eng.tensor_tensor(out=ft, in0=pt[:, cc, 0:HW],
                                     in1=pt[:, cc, HW:2 * HW], op=ALU.add)
                n = HW // 2
                while n > RHO:
                    nc.vector.tensor_tensor(out=ft[:, 0:n], in0=ft[:, 0:n],
                                            in1=ft[:, n:2 * n], op=ALU.add)
                    n //= 2
                nc.vector.tensor_tensor(
                    out=fe_sb[:, cc, :, b], in0=ft[:, 0:RHO],
                    in1=ft[:, RHO:2 * RHO], op=ALU.add)

            for b in range(BPC):
                for cc in range(CCH):
                    eng = nc.gpsimd if (b, cc) in GP_L1 else nc.vector
                    tree(eng, pts[b], cc, b)

            # polar half of the head linear + merge into pre_sb
            for r in range(RHO):
                hp = hps.tile([BPC, NH], F32, tag="hp", name=f"hpP{r}")
                for kk in range(CCH):
                    nc.tensor.matmul(hp, fe_sb[:, kk, r, :],
                                     w1_sb[:, kk, r, :],
                                     start=(kk == 0), stop=(kk == CCH - 1))
                nc.vector.tensor_tensor(out=pre_sb[:, r, :], in0=hp,
                                        in1=hpre_sb[:, r, :], op=ALU.add)

            # jacobi scan: pass 0 + NJAC correction passes on [4, 25, 40]
            nc.vector.memset(accsh, 0.0)
            t0 = scanw.tile([BPC, RHO, NH], F32, tag="t")
            nc.scalar.activation(out=t0, in_=pre_sb, func=AF.Tanh, scale=GC)
            xw0 = scanw.tile([BPC, RHO, NH], F32, tag="xw")
            nc.vector.tensor_tensor(out=xw0, in0=pre_sb, in1=w2hb_sb,
                                    op=ALU.mult)
            p0t = scanw.tile([BPC, RHO, NH], F32, tag="pp")
            nc.vector.scalar_tensor_tensor(
                out=p0t, in0=t0, scalar=1.0, in1=xw0,
                op0=ALU.add, op1=ALU.mult)
            acc = scanw.tile([BPC, RHO], F32, tag="a", name="acc0")
            nc.vector.reduce_sum(out=acc, in_=p0t, axis=AX.X)
            for it in range(NJAC):
                nc.vector.tensor_copy(out=accsh[:, 1:RHO, 0],
                                      in_=acc[:, 0:RHO - 1])
                prek = scanw.tile([BPC, RHO, NH], F32, tag="pk")
                nc.vector.tensor_tensor(
                    out=prek, in0=wrecb_sb,
                    in1=accsh.broadcast_to([BPC, RHO, NH]), op=ALU.mult)
                nc.vector.tensor_tensor(out=prek, in0=prek, in1=pre_sb,
                                        op=ALU.add)
                tk = scanw.tile([BPC, RHO, NH], F32, tag="t")
                nc.scalar.activation(out=tk, in_=prek, func=AF.Tanh, scale=GC)
                xwk = scanw.tile([BPC, RHO, NH], F32, tag="xw")
                nc.vector.tensor_tensor(out=xwk, in0=prek, in1=w2hb_sb,
                                        op=ALU.mult)
                pkt = scanw.tile([BPC, RHO, NH], F32, tag="pp")
                nc.vector.scalar_tensor_tensor(
                    out=pkt, in0=tk, scalar=1.0, in1=xwk,
                    op0=ALU.add, op1=ALU.mult)
                acc = scanw.tile([BPC, RHO], F32, tag="a", name=f"acc{it + 1}")
                nc.vector.reduce_sum(out=acc, in_=pkt, axis=AX.X)

            outv = sing.tile([BPC, RHO], F32)
            nc.vector.tensor_tensor(out=outv, in0=acc, in1=b2b_sb, op=ALU.add)
            nc.vector.tensor_scalar(out=outv, in0=outv,
                                    scalar1=0.0, scalar2=float(np.pi),
                                    op0=ALU.max, op1=ALU.min)
            nc.sync.dma_start(out=out.ap(), in_=outv)

    nc.finalize()
    return nc


def kernel(polar_feat, cart_feat, grid, W1_0, b1_0, W2_0, b2_0,
           W1s, b1s, W2s, b2s):
    global LAST_RESULTS
    f = np.float32
    polar_feat = np.ascontiguousarray(polar_feat, f)
    cart_feat = np.ascontiguousarray(cart_feat, f)
    grid = np.asarray(grid, f)

    # polar: int8 with per-core per-(c,ring) scales (folded into w1)
    s = np.abs(polar_feat).reshape(NCORES, BPC, C, RHO, WP).max(axis=(1, 4))
    s = np.maximum(s / f(127.0), f(1e-30)).astype(f)          # [8, C, 25]
    sb = np.repeat(s, BPC, axis=0)[:, :, :, None]             # [32, C, 25, 1]
    polar_q = np.clip(np.rint(polar_feat / sb), -127, 127).astype(np.int8)
    polar_p = np.ascontiguousarray(
        polar_q.reshape(B, CCH, 128, RHO, WP).transpose(0, 2, 1, 4, 3)
    ).reshape(B, 128, CCH, WP * RHO)

    # cart: fp8 e3m4, layout [B, 128(pix-sub), KCH, C]
    cart8 = cart_feat.reshape(B, C, KCH, 128).astype(ml_dtypes.float8_e3m4)
    cart_p = np.ascontiguousarray(cart8.transpose(0, 3, 2, 1))

    # smat with the /WP mean fold, bf16, [128, B, KCH, RHO]
    smat = _build_smat(grid) / f(WP)
    smat_p = np.ascontiguousarray(
        smat.astype(ml_dtypes.bfloat16).reshape(B, KCH, 128, RHO)
        .transpose(2, 0, 1, 3))

    # head weights, bf16, with the polar dequant scale and the /WP mean
    # folded into the polar half per core
    W1c = np.concatenate([np.asarray(W1_0, f)[None],
                          np.asarray(W1s, f)[:, :D, :]], 0)
    w1_cores = []
    for core in range(NCORES):
        w1q = W1c.copy()
        w1q[:, :C, :] *= (s[core].transpose(1, 0) / f(WP))[:, :, None]
        w1_cores.append(np.ascontiguousarray(
            w1q.reshape(RHO, DCH, 128, NH).transpose(2, 1, 0, 3)
            .astype(ml_dtypes.bfloat16)))

    wr = np.concatenate([np.zeros((1, NH), f), np.asarray(W1s, f)[:, D, :]], 0)
    b1 = np.concatenate([np.asarray(b1_0, f)[None], np.asarray(b1s, f)], 0)
    b2 = np.concatenate([np.asarray(b2_0, f)[None], np.asarray(b2s, f)], 0)[:, 0]
    W2 = np.concatenate([np.asarray(W2_0, f)[None], np.asarray(W2s, f)], 0)[:, :, 0]
    b1_eff = b1.copy()
    b1_eff[1:] += wr[1:] * b2[:-1, None]

    b1b = np.ascontiguousarray(b1_eff[None])                       # [1,25,40]
    wrecb = np.ascontiguousarray(np.broadcast_to(wr[None], (BPC, RHO, NH)))
    w2hb = np.ascontiguousarray(
        np.broadcast_to((W2 * f(0.5))[None], (BPC, RHO, NH)))
    b2b = np.ascontiguousarray(np.broadcast_to(b2[None], (BPC, RHO)))

    nc = _build_program()
    in_maps = []
    for core in range(NCORES):
        b0 = core * BPC
        in_maps.append({
            "polar": np.ascontiguousarray(polar_p[b0:b0 + BPC]),
            "cart": np.ascontiguousarray(cart_p[b0:b0 + BPC]).view(np.uint8),
            "smat": np.ascontiguousarray(smat_p[:, b0:b0 + BPC]).view(np.uint16),
            "w1": w1_cores[core].view(np.uint16),
            "b1b": b1b,
            "wrecb": wrecb,
            "w2hb": w2hb,
            "b2b": b2b,
        })
    res = bass_utils.run_bass_kernel_spmd(
        nc, in_maps, core_ids=list(range(NCORES)), trace=TRACE, **TRACE_KW)
    LAST_RESULTS = res
    return np.concatenate([r["out"] for r in res.results], axis=0)


# revision 17
# speedup vs baseline: 1.0639x; 1.0639x over previous
"""Trainium2 Bass kernel: polar/cartesian ConvNext feature mix + 25-head scan.

Full (unsharded) inputs in, full output out. Pure data-parallel over batch
(32 -> 4 per core x 8 cores).

v2 design (validated vs the jax reference at ~1e-2 rel in numpy sim):
  * polar stream: int8 with per-core per-(channel,ring) scales in HBM;
    the width-256 mean starts as a fold-2 SWDGE cast+accum DMA (int8
    pairs sum exactly in f16), then a flat in-place f16 halving tree on
    the DVE (w-major host layout keeps every level a packed 2-d
    TensorTensor eligible for the 2x mode). The dequant scales and the
    /256 fold into a per-core bf16 W1 (bf16 dodges f16 subnormals), so
    fe holds raw integer sums.
  * cart stream: fp8-e3m4 in HBM, consumed directly by the PE as the
    moving operand against a bf16 smat (grid-sample weight matrix, /256
    folded in) built host-side from `grid`.
  * head linear: per-ring [128,4]x[128,40] matmuls into [4,40] PSUM
    groups (engine APs require 32-aligned partition bases, so batched
    100-partition layouts are not expressible); b1_eff enters each group
    as a K=1 ones-row matmul; the cart half runs during the polar stream
    with scalar-engine PSUM evacuation, the polar half merges at the end.
  * the sequential 25-step head recurrence is replaced by Jacobi
    iteration on [4, 25, 40] tiles: the recurrent term's gain is ~1.3e-3
    per step, so pass-0 + one correction pass is numerically exact; the
    ring-shift of the carried scalar is a free-dim shifted copy.
    gelu(exact) ~= 0.5*x*(1+tanh(c*x)).
  * engine assignment: cart/const DMAs on sync (HWDGE), polar DMAs on
    gpsimd (SWDGE), PSUM evacuation copies on the scalar engine (its
    in-order stream absorbs the PE-wait stalls), all tree/scale/Jacobi
    math on the DVE, which must stay free of PE-dependent stalls.
"""
import numpy as np
import ml_dtypes

import concourse.bacc as bacc
import concourse.mybir as mybir
import concourse.tile as tile
from concourse import bass_utils
from concourse.masks import make_identity

F32 = mybir.dt.float32
F16 = mybir.dt.float16
BF16 = mybir.dt.bfloat16
F8E3 = mybir.dt.float8e3
I8 = mybir.dt.int8
AF = mybir.ActivationFunctionType
ALU = mybir.AluOpType
AX = mybir.AxisListType

# Problem shapes (fixed by the spec)
B, C, RHO, WP = 32, 384, 25, 256
HC = WC = 64
NPIX = HC * WC            # 4096
D = 2 * C                 # 768
NH = 40
NCORES = 8
BPC = B // NCORES         # 4
CCH = C // 128            # 3 channel chunks
KCH = NPIX // 128         # 32 pixel chunks
DCH = D // 128            # 6 feature chunks
KHALF = KCH // 2
NJAC = 1                  # jacobi correction passes

GC = 0.7978845608028654   # sqrt(2/pi)

TRACE = False             # test harness may flip this for profiling
TRACE_KW: dict = {}
LAST_RESULTS = None


def _build_smat(grid):
    """[B, 4096, 25] f32: summed bilinear weights per (pixel, ring).

    Index math replicates the reference exactly (f32 floor/clip)."""
    gx = grid[..., 0].astype(np.float32)
    gy = grid[..., 1].astype(np.float32)
    ix = (gx + np.float32(1.0)) * np.float32(WC * 0.5) - np.float32(0.5)
    iy = (gy + np.float32(1.0)) * np.float32(HC * 0.5) - np.float32(0.5)
    ix0 = np.floor(ix)
    iy0 = np.floor(iy)
    tx = ix - ix0
    ty = iy - iy0
    corners = (
        (ix0, iy0, (1 - tx) * (1 - ty)),
        (ix0 + 1, iy0, tx * (1 - ty)),
        (ix0, iy0 + 1, (1 - tx) * ty),
        (ix0 + 1, iy0 + 1, tx * ty),
    )
    boff = np.arange(B, dtype=np.int64)[:, None, None] * (NPIX * RHO)
    roff = np.arange(RHO, dtype=np.int64)[None, :, None]
    keys = []
    vals = []
    for xi, yi, w in corners:
        valid = (xi >= 0) & (xi < WC) & (yi >= 0) & (yi < HC)
        xc = np.clip(xi, 0, WC - 1).astype(np.int64)
        yc = np.clip(yi, 0, HC - 1).astype(np.int64)
        keys.append((boff + (yc * WC + xc) * RHO + roff).ravel())
        vals.append((w * valid).astype(np.float64).ravel())
    s = np.bincount(np.concatenate(keys), weights=np.concatenate(vals),
                    minlength=B * NPIX * RHO)
    return s.reshape(B, NPIX, RHO).astype(np.float32)


def _build_program():
    nc = bacc.Bacc("TRN2", target_bir_lowering=False, debug=False,
                   enable_asserts=False, num_devices=NCORES)
    polar = nc.dram_tensor("polar", [BPC, 128, CCH, WP * RHO], I8,
                           kind="ExternalInput")
    cart = nc.dram_tensor("cart", [BPC, 128, KCH, C], F8E3,
                          kind="ExternalInput")
    smat = nc.dram_tensor("smat", [128, BPC, KCH, RHO], BF16,
                          kind="ExternalInput")
    w1 = nc.dram_tensor("w1", [128, DCH, RHO, NH], BF16, kind="ExternalInput")
    b1b = nc.dram_tensor("b1b", [1, RHO, NH], F32, kind="ExternalInput")
    wrecb = nc.dram_tensor("wrecb", [BPC, RHO, NH], F32, kind="ExternalInput")
    w2hb = nc.dram_tensor("w2hb", [BPC, RHO, NH], F32, kind="ExternalInput")
    b2b = nc.dram_tensor("b2b", [BPC, RHO], F32, kind="ExternalInput")
    out = nc.dram_tensor("out", [BPC, RHO], F32, kind="ExternalOutput")

    with tile.TileContext(nc) as tc:
        with (
            tc.tile_pool(name="sing", bufs=1) as sing,
            tc.tile_pool(name="cpool", bufs=3) as cpool,
            tc.tile_pool(name="ppool", bufs=3) as ppool,
            tc.tile_pool(name="fpool", bufs=4) as fpool,
            tc.tile_pool(name="fcpool", bufs=2) as fcpool,
            tc.tile_pool(name="scanw", bufs=1) as scanw,
            tc.tile_pool(name="cps", bufs=2, space="PSUM") as cps,
            tc.tile_pool(name="tps", bufs=2, space="PSUM") as tps,
            tc.tile_pool(name="hps", bufs=3, space="PSUM") as hps,
        ):
            stile = sing.tile([128, BPC, KCH, RHO], BF16)
            w1_sb = sing.tile([128, DCH, RHO, NH], BF16)
            fe_sb = sing.tile([128, DCH, RHO, BPC], BF16)
            ident = sing.tile([RHO, RHO], F32)
            b1b_sb = sing.tile([1, RHO, NH], F32)
            wrecb_sb = sing.tile([BPC, RHO, NH], F32)
            w2hb_sb = sing.tile([BPC, RHO, NH], F32)
            b2b_sb = sing.tile([BPC, RHO], F32)
            onesc = sing.tile([1, BPC], F32)
            hpre_sb = sing.tile([BPC, RHO, NH], F32)
            pre_sb = sing.tile([BPC, RHO, NH], F32)
            accsh = sing.tile([BPC, RHO, 1], F32)
            warm = sing.tile([BPC, RHO], F32)

            def load_consts():
                make_identity(nc, ident)
                nc.sync.dma_start(out=stile, in_=smat.ap())
                nc.sync.dma_start(out=w1_sb, in_=w1.ap())
                nc.sync.dma_start(out=b1b_sb, in_=b1b.ap())
                nc.sync.dma_start(out=wrecb_sb, in_=wrecb.ap())
                nc.sync.dma_start(out=w2hb_sb, in_=w2hb.ap())
                nc.sync.dma_start(out=b2b_sb, in_=b2b.ap())
                nc.vector.memset(onesc, 1.0)
                # warm the tanh LUT while streaming
                nc.scalar.activation(out=warm, in_=b2b_sb, func=AF.Tanh,
                                     scale=1.0)

            # polar lands via fold-2 SWDGE cast+accum DMAs: the second
            # half-width accumulates onto the first in f16 (sums <=254 are
            # exact), halving the on-chip tree work. batch 0 per-chunk so
            # its trees start early
            HW = WP * RHO // 2
            pts = []
            for b in range(BPC):
                pt = ppool.tile([128, CCH, WP * RHO], I8, tag="p",
                                name=f"p{b}")
                if b == 0:
                    for cc in range(CCH):
                        nc.gpsimd.dma_start(out=pt[:, cc, :],
                                            in_=polar.ap()[b][:, cc])
                else:
                    nc.gpsimd.dma_start(out=pt, in_=polar.ap()[b])
                pts.append(pt)

            # cart stream + matmuls + transposes (PE) / evacuations (ACT)
            for b in range(BPC):
                ctl = cpool.tile([128, KCH, C], F8E3, tag="c")
                nc.sync.dma_start(out=ctl[:, 0:KHALF, :],
                                  in_=cart.ap()[b][:, 0:KHALF, :])
                nc.sync.dma_start(out=ctl[:, KHALF:KCH, :],
                                  in_=cart.ap()[b][:, KHALF:KCH, :])
                if b == 0:
                    load_consts()
                cpsum = cps.tile([RHO, C], F32, tag="cp", name=f"cp{b}")
                for k in range(KCH):
                    nc.tensor.matmul(cpsum, stile[:, b, k, :], ctl[:, k, :],
                                     start=(k == 0), stop=(k == KCH - 1))
                fecart = fcpool.tile([RHO, C], F32, tag="fc", name=f"fc{b}")
                nc.scalar.copy(out=fecart, in_=cpsum)
                for cc in range(CCH):
                    tp = tps.tile([128, RHO], F32, tag="tp",
                                  name=f"tp{b}_{cc}")
                    nc.tensor.transpose(
                        tp, fecart[:, cc * 128:(cc + 1) * 128], ident)
                    nc.scalar.copy(out=fe_sb[:, CCH + cc, :, b], in_=tp)

            # cart half of the head linear: per-ring [4,40] groups with
            # the b1_eff bias entering as a K=1 ones-row matmul; evacuated
            # by the scalar engine so the DVE stream stays stall-free
            for r in range(RHO):
                hp = hps.tile([BPC, NH], F32, tag="hp", name=f"hpC{r}")
                nc.tensor.matmul(hp, onesc, b1b_sb[:, r, :],
                                 start=True, stop=False)
                for kk in range(CCH, DCH):
                    nc.tensor.matmul(hp, fe_sb[:, kk, r, :],
                                     w1_sb[:, kk, r, :],
                                     start=False, stop=(kk == DCH - 1))
                nc.scalar.copy(out=hpre_sb[:, r, :], in_=hp)

            # polar reduction trees: flat contiguous in-place halving in
            # f16 (2-d packed TensorTensor hits the DVE 2x mode); the
            # final level writes the raw ring sums into fe_sb (the int8
            # dequant scales are folded into the bf16 W1 host-side)
            # level 1 (int8, always 1x) is the expensive level: offload a
            # few tiles' L1 to the otherwise-idle gpsimd engine; the f16
            # levels (2x-eligible) stay on the DVE
            def tree(pt, cc, b):
                ft = fpool.tile([128, HW], F16, tag="f")
                nc.vector.tensor_tensor(out=ft, in0=pt[:, cc, 0:HW],
                                        in1=pt[:, cc, HW:2 * HW], op=ALU.add)
                n = HW // 2
                while n > RHO:
                    nc.vector.tensor_tensor(out=ft[:, 0:n], in0=ft[:, 0:n],
                                            in1=ft[:, n:2 * n], op=ALU.add)
                    n //= 2
                nc.vector.tensor_tensor(
                    out=fe_sb[:, cc, :, b], in0=ft[:, 0:RHO],
                    in1=ft[:, RHO:2 * RHO], op=ALU.add)

            for b in range(BPC):
                for cc in range(CCH):
                    tree(pts[b], cc, b)

            # polar half of the head linear + merge into pre_sb
            for r in range(RHO):
                hp = hps.tile([BPC, NH], F32, tag="hp", name=f"hpP{r}")
                for kk in range(CCH):
                    nc.tensor.matmul(hp, fe_sb[:, kk, r, :],
                                     w1_sb[:, kk, r, :],
                                     start=(kk == 0), stop=(kk == CCH - 1))
                nc.vector.tensor_tensor(out=pre_sb[:, r, :], in0=hp,
                                        in1=hpre_sb[:, r, :], op=ALU.add)

            # jacobi scan: pass 0 + NJAC correction passes on [4, 25, 40]
            nc.vector.memset(accsh, 0.0)
            t0 = scanw.tile([BPC, RHO, NH], F32, tag="t")
            nc.scalar.activation(out=t0, in_=pre_sb, func=AF.Tanh, scale=GC)
            xw0 = scanw.tile([BPC, RHO, NH], F32, tag="xw")
            nc.vector.tensor_tensor(out=xw0, in0=pre_sb, in1=w2hb_sb,
                                    op=ALU.mult)
            p0t = scanw.tile([BPC, RHO, NH], F32, tag="pp")
            nc.vector.scalar_tensor_tensor(
                out=p0t, in0=t0, scalar=1.0, in1=xw0,
                op0=ALU.add, op1=ALU.mult)
            acc = scanw.tile([BPC, RHO], F32, tag="a", name="acc0")
            nc.vector.reduce_sum(out=acc, in_=p0t, axis=AX.X)
            for it in range(NJAC):
                nc.vector.tensor_copy(out=accsh[:, 1:RHO, 0],
                                      in_=acc[:, 0:RHO - 1])
                prek = scanw.tile([BPC, RHO, NH], F32, tag="pk")
                nc.vector.tensor_tensor(
                    out=prek, in0=wrecb_sb,
                    in1=accsh.broadcast_to([BPC, RHO, NH]), op=ALU.mult)
                nc.vector.tensor_tensor(out=prek, in0=prek, in1=pre_sb,
                                        op=ALU.add)
                tk = scanw.tile([BPC, RHO, NH], F32, tag="t")
                nc.scalar.activation(out=tk, in_=prek, func=AF.Tanh, scale=GC)
                xwk = scanw.tile([BPC, RHO, NH], F32, tag="xw")
                nc.vector.tensor_tensor(out=xwk, in0=prek, in1=w2hb_sb,
                                        op=ALU.mult)
                pkt = scanw.tile([BPC, RHO, NH], F32, tag="pp")
                nc.vector.scalar_tensor_tensor(
                    out=pkt, in0=tk, scalar=1.0, in1=xwk,
                    op0=ALU.add, op1=ALU.mult)
                acc = scanw.tile([BPC, RHO], F32, tag="a", name=f"acc{it + 1}")
                nc.vector.reduce_sum(out=acc, in_=pkt, axis=AX.X)

            outv = sing.tile([BPC, RHO], F32)
            nc.vector.tensor_tensor(out=outv, in0=acc, in1=b2b_sb, op=ALU.add)
            nc.vector.tensor_scalar(out=outv, in0=outv,
                                    scalar1=0.0, scalar2=float(np.pi),
                                    op0=ALU.max, op1=ALU.min)
            nc.sync.dma_start(out=out.ap(), in_=outv)

    nc.finalize()
    return nc


def kernel(polar_feat, cart_feat, grid, W1_0, b1_0, W2_0, b2_0,
           W1s, b1s, W2s, b2s):
    global LAST_RESULTS
    f = np.float32
    polar_feat = np.ascontiguousarray(polar_feat, f)
    cart_feat = np.ascontiguousarray(cart_feat, f)
    grid = np.asarray(grid, f)

    # polar: int8 with per-core per-(c,ring) scales (folded into w1)
    s = np.abs(polar_feat).reshape(NCORES, BPC, C, RHO, WP).max(axis=(1, 4))
    s = np.maximum(s / f(127.0), f(1e-30)).astype(f)          # [8, C, 25]
    sb = np.repeat(s, BPC, axis=0)[:, :, :, None]             # [32, C, 25, 1]
    polar_q = np.clip(np.rint(polar_feat / sb), -127, 127).astype(np.int8)
    polar_p = np.ascontiguousarray(
        polar_q.reshape(B, CCH, 128, RHO, WP).transpose(0, 2, 1, 4, 3)
    ).reshape(B, 128, CCH, WP * RHO)

    # cart: fp8 e3m4, layout [B, 128(pix-sub), KCH, C]
    cart8 = cart_feat.reshape(B, C, KCH, 128).astype(ml_dtypes.float8_e3m4)
    cart_p = np.ascontiguousarray(cart8.transpose(0, 3, 2, 1))

    # smat with the /WP mean fold, bf16, [128, B, KCH, RHO]
    smat = _build_smat(grid) / f(WP)
    smat_p = np.ascontiguousarray(
        smat.astype(ml_dtypes.bfloat16).reshape(B, KCH, 128, RHO)
        .transpose(2, 0, 1, 3))

    # head weights, bf16, with the polar dequant scale and the /WP mean
    # folded into the polar half per core
    W1c = np.concatenate([np.asarray(W1_0, f)[None],
                          np.asarray(W1s, f)[:, :D, :]], 0)
    w1_cores = []
    for core in range(NCORES):
        w1q = W1c.copy()
        w1q[:, :C, :] *= (s[core].transpose(1, 0) / f(WP))[:, :, None]
        w1_cores.append(np.ascontiguousarray(
            w1q.reshape(RHO, DCH, 128, NH).transpose(2, 1, 0, 3)
            .astype(ml_dtypes.bfloat16)))

    wr = np.concatenate([np.zeros((1, NH), f), np.asarray(W1s, f)[:, D, :]], 0)
    b1 = np.concatenate([np.asarray(b1_0, f)[None], np.asarray(b1s, f)], 0)
    b2 = np.concatenate([np.asarray(b2_0, f)[None], np.asarray(b2s, f)], 0)[:, 0]
    W2 = np.concatenate([np.asarray(W2_0, f)[None], np.asarray(W2s, f)], 0)[:, :, 0]
    b1_eff = b1.copy()
    b1_eff[1:] += wr[1:] * b2[:-1, None]

    b1b = np.ascontiguousarray(b1_eff[None])                       # [1,25,40]
    wrecb = np.ascontiguousarray(np.broadcast_to(wr[None], (BPC, RHO, NH)))
    w2hb = np.ascontiguousarray(
        np.broadcast_to((W2 * f(0.5))[None], (BPC, RHO, NH)))
    b2b = np.ascontiguousarray(np.broadcast_to(b2[None], (BPC, RHO)))

    nc = _build_program()
    in_maps = []
    for core in range(NCORES):
        b0 = core * BPC
        in_maps.append({
            "polar": np.ascontiguousarray(polar_p[b0:b0 + BPC]),
            "cart": np.ascontiguousarray(cart_p[b0:b0 + BPC]).view(np.uint8),
            "smat": np.ascontiguousarray(smat_p[:, b0:b0 + BPC]).view(np.uint16),
            "w1": w1_cores[core].view(np.uint16),
            "b1b": b1b,
            "wrecb": wrecb,
            "w2hb": w2hb,
            "b2b": b2b,
        })
    res = bass_utils.run_bass_kernel_spmd(
        nc, in_maps, core_ids=list(range(NCORES)), trace=TRACE, **TRACE_KW)
    LAST_RESULTS = res
    return np.concatenate([r["out"] for r in res.results], axis=0)
